# revision 2
# baseline (speedup 1.0000x reference)
"""Multi-head causal attention Bass kernel for Trainium2, 8-core SPMD. v6

Problem: B=2, S=2048, D=1024, H=16, DH=64.
Sharding: core c -> batch b = c // 4, head group g = c % 4 (heads 4g..4g+3).

v7 (= v6 with the racy 128-wide ctx stationary reverted to 65-wide, and
separate single-buffer psum tags for the A/B context accumulators).

v6 core idea (from HW microbenchmarks): the per-tile softmax pipeline was
serialized through shared psum/e-tile pool rotations. Splitting each
head-pair into TWO independent streams with separate single-bank psum tags
unlocks real engine parallelism:
  head A stream: scores -> psum tag scA -> ACT exp           -> ctxA
  head B stream: scores -> psum tag scB -> DVE Schraudolph   -> ctxB
(bf16 Schraudolph: e = bitcast16(int16(s*0.125*128/ln2 + (127*128-5.5))),
 ~3.3% elementwise, consistent per head so softmax bias mostly cancels.)

Also:
- normalization on HOST: y [4, 65, S] bf16 = unnormalized ctx^T + denom row
  (ones column in V at 65th output row); per-(head,chunk) output DMA.
- causal boundary masks on GPSIMD (off the ACT/DVE critical path).
- projections stream into attention as per-matmul "filler" closures popped
  one group per attention iteration (PE fills exp-bound gaps).
- ctx matmuls use a 128-wide stationary slice (padded V) so FWL engages.

PSUM: scA 2x1 + scB 2x1 + ctx 2x1 + proj 2x1 = 8 banks.
"""

import sys

import numpy as np

try:
    import concourse.bass as bass  # noqa: F401
except ImportError:
    for _p in ("/opt/trn_rl_repo", "/root/.axon_site/_ro/trn_rl_repo"):
        if _p not in sys.path:
            sys.path.insert(0, _p)
    import concourse.bass as bass  # noqa: F401

from concourse import bacc
import concourse.mybir as mybir
import concourse.tile as tile

F32 = mybir.dt.float32
BF16 = mybir.dt.bfloat16
I16 = mybir.dt.int16

S = 2048          # sequence length
D = 1024          # model dim (contraction for projections)
HPC = 4           # heads per core
DH = 64           # head dim
NK = D // 128     # 8 contraction chunks
NST = S // 128    # 16 sequence tiles of 128
NCH = S // 512    # 4 s-chunks of 512
VW = HPC * (DH + 1)   # 260 cols per V s-tile

# bf16 Schraudolph exp of (0.125 * s)
SCH_C1 = 0.125 * 128.0 / 0.6931471805599453
SCH_C2 = 127.0 * 128.0 - 5.5


def build_kernel(loop_n=0):
    nc = bacc.Bacc("TRN2", target_bir_lowering=False, debug=True)

    xT = nc.dram_tensor("xT", [D, S], BF16, kind="ExternalInput")
    wq = nc.dram_tensor("wq", [D, HPC * DH], BF16, kind="ExternalInput")
    wk = nc.dram_tensor("wk", [D, HPC * DH], BF16, kind="ExternalInput")
    wv = nc.dram_tensor("wv", [D, HPC * DH], BF16, kind="ExternalInput")
    mask_in = nc.dram_tensor("mask_in", [128, 512], BF16,
                             kind="ExternalInput")
    y = nc.dram_tensor("y", [HPC, DH + 1, S], BF16, kind="ExternalOutput")

    with tile.TileContext(nc) as tc:
        from contextlib import ExitStack
        stk = ExitStack()
        loop = stk.enter_context(tc.For_i(0, loop_n, 1)) if loop_n else None
        with stk, (
            tc.tile_pool(name="persist", bufs=1)
        ) as pers, (
            tc.tile_pool(name="proj_ps", bufs=2, space="PSUM")
        ) as proj_ps, (
            tc.tile_pool(name="score_ps", bufs=2, space="PSUM")
        ) as score_ps, (
            tc.tile_pool(name="ctx_ps", bufs=2, space="PSUM")
        ) as ctx_ps_pool, (
            tc.tile_pool(name="esb", bufs=4)
        ) as esb_pool:
            # ---- persistent SBUF tiles -------------------------------------
            xT0 = [
                pers.tile([128, 4 * 512], BF16, tag=f"xT0{hh}", name=f"xT0{hh}")
                for hh in range(2)
            ]
            xT_ch = [None] + [
                pers.tile([128, NK * 512], BF16, tag=f"xTc{ch}", name=f"xTc{ch}")
                for ch in range(1, NCH)
            ]
            w_all = {
                wname: pers.tile([128, NK * HPC * DH], BF16, name=f"w_{wname}")
                for wname in ("q", "k", "v")
            }
            QT_sb = [
                [pers.tile([128, 512], BF16, tag=f"QT{m}c{ch}",
                           name=f"QT{m}c{ch}") for ch in range(NCH)]
                for m in range(2)
            ]
            KT_sb = [
                [pers.tile([128, 512], BF16, tag=f"KT{m}c{ch}",
                           name=f"KT{m}c{ch}") for ch in range(NCH)]
                for m in range(2)
            ]
            V4 = [
                pers.tile([128, 4 * VW], BF16, tag=f"V4_{q}", name=f"V4_{q}")
                for q in range(4)
            ]
            y_sb = [
                pers.tile([DH + 1, S], BF16, tag=f"y{h}", name=f"y{h}")
                for h in range(HPC)
            ]
            mask2 = pers.tile([128, 512], BF16, tag="mask2", name="mask2")

            def xs(ch, kk):      # xT chunk ch, k-chunk kk -> [128, 512]
                if ch == 0:
                    return xT0[kk // 4][:, 512 * (kk % 4):512 * (kk % 4 + 1)]
                return xT_ch[ch][:, 512 * kk:512 * (kk + 1)]

            def ws(wname, kk):   # w k-chunk [128, 256]
                return w_all[wname][:, HPC * DH * kk:HPC * DH * (kk + 1)]

            def vs(t):           # V s-tile t -> [128, 260]
                return V4[t // 4][:, VW * (t % 4):VW * (t % 4 + 1)]

            # ---- input DMAs ------------------------------------------------
            xTr = xT.rearrange("(k p) (c s) -> p c k s", k=NK, c=NCH)
            for hh in range(2):
                nc.scalar.dma_start(
                    out=xT0[hh].rearrange("p (k s) -> p k s", k=4),
                    in_=xTr[:, 0, 4 * hh:4 * (hh + 1)],
                )
            for wname, wdram in (("q", wq), ("k", wk)):
                nc.sync.dma_start(
                    out=w_all[wname].rearrange("p (k e) -> p k e", k=NK),
                    in_=wdram.rearrange("(k p) e -> p k e", k=NK),
                )
            nc.sync.dma_start(out=mask2[:], in_=mask_in[:])
            for q in range(4):
                for tt in range(4):
                    for h in range(HPC):
                        nc.vector.memset(
                            V4[q][:, VW * tt + (DH + 1) * h + DH:
                                  VW * tt + (DH + 1) * h + DH + 1], 1.0)
            nc.sync.dma_start(
                out=w_all["v"].rearrange("p (k e) -> p k e", k=NK),
                in_=wv.rearrange("(k p) e -> p k e", k=NK),
            )
            for ch in range(1, NCH):
                nc.scalar.dma_start(
                    out=xT_ch[ch].rearrange("p (k s) -> p k s", k=NK),
                    in_=xTr[:, ch],
                )
            mask3 = mask2.rearrange("p (h i) -> p h i", h=2)

            # ---- projection emitters (per-matmul filler closures) ----------
            def emit_proj_qk(ch, m, wname):
                dest = QT_sb if wname == "q" else KT_sb
                state = {}
                def step(kk, state=state):
                    if kk == 0:
                        state["ps"] = proj_ps.tile([128, 512], F32, tag="proj",
                                                   name="ps_qk")
                    nc.tensor.matmul(
                        state["ps"][:],
                        ws(wname, kk)[:, 128 * m:128 * (m + 1)],
                        xs(ch, kk),
                        start=(kk == 0),
                        stop=(kk == NK - 1),
                    )
                    if kk == NK - 1:
                        nc.scalar.copy(dest[m][ch][:], state["ps"][:])
                return [lambda kk=kk: step(kk) for kk in range(NK)]

            def emit_proj_v(t):
                state = {}
                def step(kk, state=state):
                    if kk == 0:
                        state["ps"] = proj_ps.tile([128, HPC * DH], F32,
                                                   tag="proj", name="ps_v")
                    nc.tensor.matmul(
                        state["ps"][:],
                        xs(t // 4, kk)[:, 128 * (t % 4):128 * (t % 4 + 1)],
                        ws("v", kk),
                        start=(kk == 0),
                        stop=(kk == NK - 1),
                    )
                    if kk == NK - 1:
                        for h in range(HPC):
                            nc.vector.tensor_copy(
                                vs(t)[:, (DH + 1) * h:(DH + 1) * h + DH],
                                state["ps"][:, DH * h:DH * (h + 1)],
                            )
                return [lambda kk=kk: step(kk) for kk in range(NK)]

            def qk_fillers(ch):
                out = []
                for m in range(2):
                    for w in ("q", "k"):
                        out.extend(emit_proj_qk(ch, m, w))
                return out

            def v_fillers(ch):
                out = []
                for t in range(4 * ch, 4 * ch + 4):
                    out.extend(emit_proj_v(t))
                return out

            vq = []        # V-proj fillers: MUST drain before V4[c] reads
            qq = []        # QK-proj fillers for chunk c+1: drain by chunk end
            slots_left = [1]
            v_deadline = [1]   # pair-0 iterations left until first V4[c] read

            def pop_filler():
                if vq:
                    n = -(-len(vq) // max(v_deadline[0], 1))
                    for _ in range(min(n, len(vq))):
                        vq.pop(0)()
                if not vq and qq:
                    n = -(-len(qq) // max(slots_left[0], 1))
                    for _ in range(min(n, len(qq))):
                        qq.pop(0)()
                slots_left[0] -= 1
                v_deadline[0] -= 1

            # ---- attention -------------------------------------------------
            e_sbs = {}

            def lo_of(c, jt):
                r = jt - 4 * c
                return 128 * r if r > 0 else 0

            def emit_scores(key):
                """Two independent head streams: A -> ACT exp, B -> DVE
                Schraudolph; separate single-bank psums + e pools."""
                pair, c, jt = key
                m = pair
                lo = lo_of(c, jt)
                psA = score_ps.tile([128, 512], F32, tag="scA", name="sA")
                psB = score_ps.tile([128, 512], F32, tag="scB", name="sB")
                for half, pp in ((0, psA), (1, psB)):
                    off = half * 64
                    nc.tensor.matmul(
                        pp[:, lo:512],
                        KT_sb[m][jt // 4][off:off + 64,
                                          128 * (jt % 4):128 * (jt % 4 + 1)],
                        QT_sb[m][c][off:off + 64, lo:512],
                        start=True,
                        stop=True,
                        tile_position=(off, 0),
                    )
                eA = esb_pool.tile([128, 512], BF16, tag="eA", name="eA")
                eB = esb_pool.tile([128, 512], BF16, tag="eB", name="eB")
                nc.scalar.activation(
                    out=eA[:, lo:512], in_=psA[:, lo:512],
                    func=mybir.ActivationFunctionType.Exp, scale=0.125,
                )
                nc.vector.tensor_scalar(
                    out=eB[:, lo:512].bitcast(I16),
                    in0=psB[:, lo:512],
                    scalar1=SCH_C1,
                    scalar2=SCH_C2,
                    op0=mybir.AluOpType.mult,
                    op1=mybir.AluOpType.add,
                )
                if jt >= 4 * c:
                    for e in (eA, eB):
                        nc.vector.tensor_mul(
                            e[:, lo:lo + 128],
                            e[:, lo:lo + 128],
                            mask3[:, 0, 128:256],
                        )
                e_sbs[key] = (eA, eB)

            def emit_ctx(pair, c, jt, ctxA, ctxB, njt):
                lo = lo_of(c, jt)
                eA, eB = e_sbs.pop((pair, c, jt))
                for half, cps, e in ((0, ctxA, eA), (1, ctxB, eB)):
                    h = 2 * pair + half
                    nc.tensor.matmul(
                        cps[:, lo:512],
                        vs(jt)[:, (DH + 1) * h:(DH + 1) * (h + 1)],
                        e[:, lo:512],
                        start=(jt == 0),
                        stop=(jt == njt - 1),
                    )

            def emit_out(pair, c, ctxA, ctxB):
                for half, cps in ((0, ctxA), (1, ctxB)):
                    h = 2 * pair + half
                    eng = nc.scalar if half == 0 else nc.vector
                    if half == 0:
                        eng.copy(y_sb[h][:, 512 * c:512 * (c + 1)], cps[:])
                    else:
                        eng.tensor_copy(y_sb[h][:, 512 * c:512 * (c + 1)],
                                        cps[:])
                    nc.sync.dma_start(
                        out=y[h][:, 512 * c:512 * (c + 1)],
                        in_=y_sb[h][:, 512 * c:512 * (c + 1)],
                    )

            def attention_pair(pair, c):
                njt = 4 * (c + 1)
                emit_scores((pair, c, 0))
                ctxA = ctx_ps_pool.tile([DH + 1, 512], F32, tag="ctxA",
                                        name="ctx_psA", bufs=1)
                ctxB = ctx_ps_pool.tile([DH + 1, 512], F32, tag="ctxB",
                                        name="ctx_psB", bufs=1)
                for jt in range(njt):
                    if jt + 1 < njt:
                        emit_scores((pair, c, jt + 1))
                    emit_ctx(pair, c, jt, ctxA, ctxB, njt)
                    pop_filler()
                emit_out(pair, c, ctxA, ctxB)

            # chunk 0 projections emitted directly
            for f in qk_fillers(0):
                f()
            for f in v_fillers(0):
                f()

            for c in range(NCH):
                if c > 0:
                    vq.extend(v_fillers(c))
                if c + 1 < NCH:
                    qq.extend(qk_fillers(c + 1))
                slots_left[0] = 2 * 4 * (c + 1)
                v_deadline[0] = max(4 * c - 1, 1)
                attention_pair(0, c)
                attention_pair(1, c)
                while vq:
                    vq.pop(0)()
                while qq:
                    qq.pop(0)()
    nc.compile()
    return nc


_CACHED = None


def get_nc():
    global _CACHED
    if _CACHED is None:
        _CACHED = build_kernel()
    return _CACHED


def _make_mask():
    """[128, 512] bf16: cols [128,256) and [384,512) hold the boundary
    mask M[dj, i''] = 1 if i'' >= dj (keep at-or-below diagonal)."""
    import ml_dtypes
    m = np.zeros((128, 512), np.float32)
    tri = np.triu(np.ones((128, 128), np.float32))
    m[:, 128:256] = tri
    m[:, 384:512] = tri
    return m.astype(ml_dtypes.bfloat16)


def shard_inputs(x, W_query, W_key, W_value):
    """Full inputs -> per-core input maps (bf16 on the wire)."""
    import ml_dtypes
    bf = ml_dtypes.bfloat16
    in_maps = []
    mask = _make_mask()
    xT_by_batch = [np.ascontiguousarray(x[b].T.astype(bf)) for b in range(2)]
    Wq16 = W_query.astype(bf)
    Wk16 = W_key.astype(bf)
    Wv16 = W_value.astype(bf)
    for core in range(8):
        b, g = core // 4, core % 4
        sl = slice(256 * g, 256 * (g + 1))
        in_maps.append({
            "xT": xT_by_batch[b],
            "wq": np.ascontiguousarray(Wq16[:, sl]),
            "wk": np.ascontiguousarray(Wk16[:, sl]),
            "wv": np.ascontiguousarray(Wv16[:, sl]),
            "mask_in": mask,
        })
    return in_maps


def assemble_output(results):
    """Per-core y [4, 65, S] bf16 (unnormalized ctx^T + denom row) ->
    full [2, S, 1024] fp32."""
    out = np.empty((2, S, 1024), np.float32)
    for core in range(8):
        b, g = core // 4, core % 4
        yv = np.asarray(results[core]["y"], dtype=np.float32)  # [4, 65, S]
        ctx = yv[:, 0:DH, :]
        den = yv[:, DH, :]
        norm = ctx / den[:, None, :]
        blk = norm.transpose(2, 0, 1).reshape(S, HPC * DH)
        out[b, :, 256 * g:256 * (g + 1)] = blk
    return out


def kernel(x, W_query, W_key, W_value):
    """Full inputs in, full output out; 8-core SPMD underneath."""
    from concourse.bass_utils import run_bass_kernel_spmd

    x = np.ascontiguousarray(np.asarray(x, dtype=np.float32))
    W_query = np.ascontiguousarray(np.asarray(W_query, dtype=np.float32))
    W_key = np.ascontiguousarray(np.asarray(W_key, dtype=np.float32))
    W_value = np.ascontiguousarray(np.asarray(W_value, dtype=np.float32))

    nc = get_nc()
    in_maps = shard_inputs(x, W_query, W_key, W_value)
    last_err = None
    for _attempt in range(3):
        try:
            res = run_bass_kernel_spmd(nc, in_maps, core_ids=list(range(8)))
            return assemble_output(res.results)
        except Exception as e:  # transient device wedges seen on this fabric
            last_err = e
            import time as _time
            _time.sleep(2.0)
    raise last_err


# revision 3
# speedup vs baseline: 1.0011x; 1.0011x over previous
"""Multi-head causal attention Bass kernel for Trainium2, 8-core SPMD. v6

Problem: B=2, S=2048, D=1024, H=16, DH=64.
Sharding: core c -> batch b = c // 4, head group g = c % 4 (heads 4g..4g+3).

v7 (= v6 with the racy 128-wide ctx stationary reverted to 65-wide, and
separate single-buffer psum tags for the A/B context accumulators).

v6 core idea (from HW microbenchmarks): the per-tile softmax pipeline was
serialized through shared psum/e-tile pool rotations. Splitting each
head-pair into TWO independent streams with separate single-bank psum tags
unlocks real engine parallelism:
  head A stream: scores -> psum tag scA -> ACT exp           -> ctxA
  head B stream: scores -> psum tag scB -> DVE Schraudolph   -> ctxB
(bf16 Schraudolph: e = bitcast16(int16(s*0.125*128/ln2 + (127*128-5.5))),
 ~3.3% elementwise, consistent per head so softmax bias mostly cancels.)

Also:
- normalization on HOST: y [4, 65, S] bf16 = unnormalized ctx^T + denom row
  (ones column in V at 65th output row); per-(head,chunk) output DMA.
- causal boundary masks on GPSIMD (off the ACT/DVE critical path).
- projections stream into attention as per-matmul "filler" closures popped
  one group per attention iteration (PE fills exp-bound gaps).
- ctx matmuls use a 128-wide stationary slice (padded V) so FWL engages.

PSUM: scA 2x1 + scB 2x1 + ctx 2x1 + proj 2x1 = 8 banks.
"""

import sys

import numpy as np

try:
    import concourse.bass as bass  # noqa: F401
except ImportError:
    for _p in ("/opt/trn_rl_repo", "/root/.axon_site/_ro/trn_rl_repo"):
        if _p not in sys.path:
            sys.path.insert(0, _p)
    import concourse.bass as bass  # noqa: F401

from concourse import bacc
import concourse.mybir as mybir
import concourse.tile as tile

F32 = mybir.dt.float32
BF16 = mybir.dt.bfloat16
I16 = mybir.dt.int16

S = 2048          # sequence length
D = 1024          # model dim (contraction for projections)
HPC = 4           # heads per core
DH = 64           # head dim
NK = D // 128     # 8 contraction chunks
NST = S // 128    # 16 sequence tiles of 128
NCH = S // 512    # 4 s-chunks of 512
VW = HPC * (DH + 1)   # 260 cols per V s-tile

# bf16 Schraudolph exp of (0.125 * s)
SCH_C1 = 0.125 * 128.0 / 0.6931471805599453
SCH_C2 = 127.0 * 128.0 - 5.5


def build_kernel(loop_n=0):
    nc = bacc.Bacc("TRN2", target_bir_lowering=False, debug=True)

    xT = nc.dram_tensor("xT", [D, S], BF16, kind="ExternalInput")
    wq = nc.dram_tensor("wq", [D, HPC * DH], BF16, kind="ExternalInput")
    wk = nc.dram_tensor("wk", [D, HPC * DH], BF16, kind="ExternalInput")
    wv = nc.dram_tensor("wv", [D, HPC * DH], BF16, kind="ExternalInput")
    mask_in = nc.dram_tensor("mask_in", [128, 512], BF16,
                             kind="ExternalInput")
    y = nc.dram_tensor("y", [HPC, DH + 1, S], BF16, kind="ExternalOutput")

    with tile.TileContext(nc) as tc:
        from contextlib import ExitStack
        stk = ExitStack()
        loop = stk.enter_context(tc.For_i(0, loop_n, 1)) if loop_n else None
        with stk, (
            tc.tile_pool(name="persist", bufs=1)
        ) as pers, (
            tc.tile_pool(name="proj_ps", bufs=2, space="PSUM")
        ) as proj_ps, (
            tc.tile_pool(name="score_ps", bufs=2, space="PSUM")
        ) as score_ps, (
            tc.tile_pool(name="ctx_ps", bufs=2, space="PSUM")
        ) as ctx_ps_pool, (
            tc.tile_pool(name="esb", bufs=4)
        ) as esb_pool:
            # ---- persistent SBUF tiles -------------------------------------
            xT0 = [
                pers.tile([128, 4 * 512], BF16, tag=f"xT0{hh}", name=f"xT0{hh}")
                for hh in range(2)
            ]
            xT_ch = [None] + [
                pers.tile([128, NK * 512], BF16, tag=f"xTc{ch}", name=f"xTc{ch}")
                for ch in range(1, NCH)
            ]
            w_all = {
                wname: pers.tile([128, NK * HPC * DH], BF16, name=f"w_{wname}")
                for wname in ("q", "k", "v")
            }
            QT_sb = [
                [pers.tile([128, 512], BF16, tag=f"QT{m}c{ch}",
                           name=f"QT{m}c{ch}") for ch in range(NCH)]
                for m in range(2)
            ]
            KT_sb = [
                [pers.tile([128, 512], BF16, tag=f"KT{m}c{ch}",
                           name=f"KT{m}c{ch}") for ch in range(NCH)]
                for m in range(2)
            ]
            V4 = [
                pers.tile([128, 4 * VW], BF16, tag=f"V4_{q}", name=f"V4_{q}")
                for q in range(4)
            ]
            y_sb = [
                pers.tile([DH + 1, S], BF16, tag=f"y{h}", name=f"y{h}")
                for h in range(HPC)
            ]
            mask2 = pers.tile([128, 512], BF16, tag="mask2", name="mask2")

            def xs(ch, kk):      # xT chunk ch, k-chunk kk -> [128, 512]
                if ch == 0:
                    return xT0[kk // 4][:, 512 * (kk % 4):512 * (kk % 4 + 1)]
                return xT_ch[ch][:, 512 * kk:512 * (kk + 1)]

            def ws(wname, kk):   # w k-chunk [128, 256]
                return w_all[wname][:, HPC * DH * kk:HPC * DH * (kk + 1)]

            def vs(t):           # V s-tile t -> [128, 260]
                return V4[t // 4][:, VW * (t % 4):VW * (t % 4 + 1)]

            # ---- input DMAs ------------------------------------------------
            xTr = xT.rearrange("(k p) (c s) -> p c k s", k=NK, c=NCH)
            for hh in range(2):
                nc.scalar.dma_start(
                    out=xT0[hh].rearrange("p (k s) -> p k s", k=4),
                    in_=xTr[:, 0, 4 * hh:4 * (hh + 1)],
                )
            for wname, wdram in (("q", wq), ("k", wk)):
                nc.sync.dma_start(
                    out=w_all[wname].rearrange("p (k e) -> p k e", k=NK),
                    in_=wdram.rearrange("(k p) e -> p k e", k=NK),
                )
            nc.sync.dma_start(out=mask2[:], in_=mask_in[:])
            for q in range(4):
                for tt in range(4):
                    for h in range(HPC):
                        nc.vector.memset(
                            V4[q][:, VW * tt + (DH + 1) * h + DH:
                                  VW * tt + (DH + 1) * h + DH + 1], 1.0)
            nc.sync.dma_start(
                out=w_all["v"].rearrange("p (k e) -> p k e", k=NK),
                in_=wv.rearrange("(k p) e -> p k e", k=NK),
            )
            for ch in range(1, NCH):
                nc.scalar.dma_start(
                    out=xT_ch[ch].rearrange("p (k s) -> p k s", k=NK),
                    in_=xTr[:, ch],
                )
            mask3 = mask2.rearrange("p (h i) -> p h i", h=2)

            # ---- projection emitters (per-matmul filler closures) ----------
            def emit_proj_qk(ch, m, wname):
                dest = QT_sb if wname == "q" else KT_sb
                state = {}
                def step(kk, state=state):
                    if kk == 0:
                        state["ps"] = proj_ps.tile([128, 512], F32, tag="proj",
                                                   name="ps_qk")
                    nc.tensor.matmul(
                        state["ps"][:],
                        ws(wname, kk)[:, 128 * m:128 * (m + 1)],
                        xs(ch, kk),
                        start=(kk == 0),
                        stop=(kk == NK - 1),
                    )
                    if kk == NK - 1:
                        nc.scalar.copy(dest[m][ch][:], state["ps"][:])
                return [lambda kk=kk: step(kk) for kk in range(NK)]

            def emit_proj_v(t):
                state = {}
                def step(kk, state=state):
                    if kk == 0:
                        state["ps"] = proj_ps.tile([128, HPC * DH], F32,
                                                   tag="proj", name="ps_v")
                    nc.tensor.matmul(
                        state["ps"][:],
                        xs(t // 4, kk)[:, 128 * (t % 4):128 * (t % 4 + 1)],
                        ws("v", kk),
                        start=(kk == 0),
                        stop=(kk == NK - 1),
                    )
                    if kk == NK - 1:
                        for h in range(HPC):
                            nc.vector.tensor_copy(
                                vs(t)[:, (DH + 1) * h:(DH + 1) * h + DH],
                                state["ps"][:, DH * h:DH * (h + 1)],
                            )
                return [lambda kk=kk: step(kk) for kk in range(NK)]

            def qk_fillers(ch):
                out = []
                for m in range(2):
                    for w in ("q", "k"):
                        out.extend(emit_proj_qk(ch, m, w))
                return out

            def v_fillers(ch):
                out = []
                for t in range(4 * ch, 4 * ch + 4):
                    out.extend(emit_proj_v(t))
                return out

            vq = []        # V-proj fillers: MUST drain before V4[c] reads
            qq = []        # QK-proj fillers for chunk c+1: drain by chunk end
            slots_left = [1]
            v_deadline = [1]   # pair-0 iterations left until first V4[c] read

            def pop_filler():
                if vq:
                    n = -(-len(vq) // max(v_deadline[0], 1))
                    for _ in range(min(n, len(vq))):
                        vq.pop(0)()
                if not vq and qq:
                    n = -(-len(qq) // max(slots_left[0], 1))
                    for _ in range(min(n, len(qq))):
                        qq.pop(0)()
                slots_left[0] -= 1
                v_deadline[0] -= 1

            # ---- attention -------------------------------------------------
            e_sbs = {}

            def lo_of(c, jt):
                r = jt - 4 * c
                return 128 * r if r > 0 else 0

            def emit_scores(key):
                """Two independent head streams: A -> ACT exp, B -> DVE
                Schraudolph; separate single-bank psums + e pools."""
                pair, c, jt = key
                m = pair
                lo = lo_of(c, jt)
                psA = score_ps.tile([128, 512], F32, tag="scA", name="sA")
                psB = score_ps.tile([128, 512], F32, tag="scB", name="sB")
                for half, pp in ((0, psA), (1, psB)):
                    off = half * 64
                    nc.tensor.matmul(
                        pp[:, lo:512],
                        KT_sb[m][jt // 4][off:off + 64,
                                          128 * (jt % 4):128 * (jt % 4 + 1)],
                        QT_sb[m][c][off:off + 64, lo:512],
                        start=True,
                        stop=True,
                        tile_position=(off, 0),
                    )
                eA = esb_pool.tile([128, 512], BF16, tag="eA", name="eA")
                eB = esb_pool.tile([128, 512], BF16, tag="eB", name="eB")
                nc.scalar.activation(
                    out=eA[:, lo:512], in_=psA[:, lo:512],
                    func=mybir.ActivationFunctionType.Exp, scale=0.125,
                )
                nc.vector.tensor_scalar(
                    out=eB[:, lo:512].bitcast(I16),
                    in0=psB[:, lo:512],
                    scalar1=SCH_C1,
                    scalar2=SCH_C2,
                    op0=mybir.AluOpType.mult,
                    op1=mybir.AluOpType.add,
                )
                if jt >= 4 * c:
                    for e in (eA, eB):
                        nc.vector.tensor_mul(
                            e[:, lo:lo + 128],
                            e[:, lo:lo + 128],
                            mask3[:, 0, 128:256],
                        )
                e_sbs[key] = (eA, eB)

            def emit_ctx(pair, c, jt, ctxA, ctxB, njt):
                lo = lo_of(c, jt)
                eA, eB = e_sbs.pop((pair, c, jt))
                for half, cps, e in ((0, ctxA, eA), (1, ctxB, eB)):
                    h = 2 * pair + half
                    nc.tensor.matmul(
                        cps[:, lo:512],
                        vs(jt)[:, (DH + 1) * h:(DH + 1) * (h + 1)],
                        e[:, lo:512],
                        start=(jt == 0),
                        stop=(jt == njt - 1),
                    )

            def emit_out(pair, c, ctxA, ctxB):
                for half, cps in ((0, ctxA), (1, ctxB)):
                    h = 2 * pair + half
                    eng = nc.scalar if half == 0 else nc.vector
                    if half == 0:
                        eng.copy(y_sb[h][:, 512 * c:512 * (c + 1)], cps[:])
                    else:
                        eng.tensor_copy(y_sb[h][:, 512 * c:512 * (c + 1)],
                                        cps[:])
                    nc.sync.dma_start(
                        out=y[h][:, 512 * c:512 * (c + 1)],
                        in_=y_sb[h][:, 512 * c:512 * (c + 1)],
                    )

            def attention_pair(pair, c):
                njt = 4 * (c + 1)
                emit_scores((pair, c, 0))
                ctxA = ctx_ps_pool.tile([DH + 1, 512], F32, tag="ctxA",
                                        name="ctx_psA", bufs=1)
                ctxB = ctx_ps_pool.tile([DH + 1, 512], F32, tag="ctxB",
                                        name="ctx_psB", bufs=1)
                for jt in range(njt):
                    if jt + 1 < njt:
                        emit_scores((pair, c, jt + 1))
                    emit_ctx(pair, c, jt, ctxA, ctxB, njt)
                    pop_filler()
                emit_out(pair, c, ctxA, ctxB)

            # chunk 0 projections emitted directly
            for f in qk_fillers(0):
                f()
            for f in v_fillers(0):
                f()

            for c in range(NCH):
                if c > 0:
                    vq.extend(v_fillers(c))
                if c + 1 < NCH:
                    qq.extend(qk_fillers(c + 1))
                slots_left[0] = 2 * 4 * (c + 1)
                v_deadline[0] = 4 * c + 1
                attention_pair(0, c)
                attention_pair(1, c)
                while vq:
                    vq.pop(0)()
                while qq:
                    qq.pop(0)()
    nc.compile()
    return nc


_CACHED = None


def get_nc():
    global _CACHED
    if _CACHED is None:
        _CACHED = build_kernel()
    return _CACHED


def _make_mask():
    """[128, 512] bf16: cols [128,256) and [384,512) hold the boundary
    mask M[dj, i''] = 1 if i'' >= dj (keep at-or-below diagonal)."""
    import ml_dtypes
    m = np.zeros((128, 512), np.float32)
    tri = np.triu(np.ones((128, 128), np.float32))
    m[:, 128:256] = tri
    m[:, 384:512] = tri
    return m.astype(ml_dtypes.bfloat16)


def shard_inputs(x, W_query, W_key, W_value):
    """Full inputs -> per-core input maps (bf16 on the wire)."""
    import ml_dtypes
    bf = ml_dtypes.bfloat16
    in_maps = []
    mask = _make_mask()
    xT_by_batch = [np.ascontiguousarray(x[b].T.astype(bf)) for b in range(2)]
    Wq16 = W_query.astype(bf)
    Wk16 = W_key.astype(bf)
    Wv16 = W_value.astype(bf)
    for core in range(8):
        b, g = core // 4, core % 4
        sl = slice(256 * g, 256 * (g + 1))
        in_maps.append({
            "xT": xT_by_batch[b],
            "wq": np.ascontiguousarray(Wq16[:, sl]),
            "wk": np.ascontiguousarray(Wk16[:, sl]),
            "wv": np.ascontiguousarray(Wv16[:, sl]),
            "mask_in": mask,
        })
    return in_maps


def assemble_output(results):
    """Per-core y [4, 65, S] bf16 (unnormalized ctx^T + denom row) ->
    full [2, S, 1024] fp32."""
    out = np.empty((2, S, 1024), np.float32)
    for core in range(8):
        b, g = core // 4, core % 4
        yv = np.asarray(results[core]["y"], dtype=np.float32)  # [4, 65, S]
        ctx = yv[:, 0:DH, :]
        den = yv[:, DH, :]
        norm = ctx / den[:, None, :]
        blk = norm.transpose(2, 0, 1).reshape(S, HPC * DH)
        out[b, :, 256 * g:256 * (g + 1)] = blk
    return out


def kernel(x, W_query, W_key, W_value):
    """Full inputs in, full output out; 8-core SPMD underneath."""
    from concourse.bass_utils import run_bass_kernel_spmd

    x = np.ascontiguousarray(np.asarray(x, dtype=np.float32))
    W_query = np.ascontiguousarray(np.asarray(W_query, dtype=np.float32))
    W_key = np.ascontiguousarray(np.asarray(W_key, dtype=np.float32))
    W_value = np.ascontiguousarray(np.asarray(W_value, dtype=np.float32))

    nc = get_nc()
    in_maps = shard_inputs(x, W_query, W_key, W_value)
    last_err = None
    for _attempt in range(3):
        try:
            res = run_bass_kernel_spmd(nc, in_maps, core_ids=list(range(8)))
            return assemble_output(res.results)
        except Exception as e:  # transient device wedges seen on this fabric
            last_err = e
            import time as _time
            _time.sleep(2.0)
    raise last_err
